# revision 7
# baseline (speedup 1.0000x reference)
"""Trainium2 Bass kernel for nn_MemoryAttention (causal single-head attention
with SiLU-gated output projection), sequence-parallel across 8 NeuronCores.

Strategy (per core c), v3:
  - q rows owned: 4 slots of 256 rows: tile t = c + 8*s (strided assignment
    balances causal work; every core runs an identical instruction stream).
  - K^T is kept SBUF-resident in fp8e4 (packed [d/256, 2, kv] for DoubleRow
    matmuls); Q^T likewise fp8-packed. wq/wk are pre-scaled x8 on the host so
    q/k values sit in fp8e4's normal range; the exp() activation scale absorbs
    the x64 on the logits.
  - KV distribution: 4 gather groups of 16 kv blocks each. Core c projects
    K^T/V only for its own tile in each group, then (K_g, V_g) AllGathers are
    triggered progressively (K before V so logits can start first).
  - kv blocks 0..3 are projected redundantly on every core (dup prefix) so
    slot 0 can start before gather A lands. V for the dup prefix stays in
    SBUF; gathered V streams from shared DRAM per visit (bf16).
  - Per (slot, kv-block) visit: LT[kv, q] = K @ QT accumulated in PSUM via
    4 DoubleRow fp8 matmuls (256-deep contraction each), PT = exp(LT*scale)
    (* mask for the last 16 visits of each slot), then PT q-chunks become the
    stationary operand for H[q, d] += P @ V and rowsums += P @ 1. The visit
    pipeline runs 2 deep and carries across slot boundaries so the PE stays
    busy through the epilogue's scalar/vector latency.
  - Slot epilogue: H / sums, SiLU via exp (no activation-table swap:
    silu(x) = x / (1 + exp(-x))), PE-transpose of G, output projection with
    G^T chunks stationary -> O[q, d] written directly.
  - Queues: visit V streams on sync; masks + staging on scalar (HWDGE);
    collectives + bulk K^T gather->SBUF loads on gpsimd.
"""

import numpy as np
import ml_dtypes

import concourse.bass as bass
import concourse.tile as tile
from concourse import bacc, mybir
from concourse.bass_utils import run_bass_kernel_spmd
from concourse.masks import make_identity

P = 128
D = 1024
SEQ = 8192
NCORES = 8
NSLOTS = 4
QT_COLS = NSLOTS * 256
NDUP = 4              # dup kv blocks, cols 0..NDUP*128
NGRP = 4              # gather groups of 16 kv blocks
N_MASKED = NSLOTS * 16
LEAD = 2              # visit software-pipeline depth

F32 = mybir.dt.float32
BF16 = mybir.dt.bfloat16
F8 = mybir.dt.float8e4
AF = mybir.ActivationFunctionType
DR = mybir.MatmulPerfMode.DoubleRow

QK_SCALE = 8.0
EXP_SCALE = 0.03125 / (QK_SCALE * QK_SCALE)

USE_DR = True


def build_kernel(use_dr=USE_DR):
    nc = bacc.Bacc(None, target_bir_lowering=False, num_devices=NCORES)

    xq_ext = nc.declare_dram_parameter("xq", [D, QT_COLS], BF16, isOutput=False)
    xd_ext = nc.declare_dram_parameter("xd", [D, NDUP * P], BF16, isOutput=False)
    wq_ext = nc.declare_dram_parameter("wq", [D, D], BF16, isOutput=False)
    wk_ext = nc.declare_dram_parameter("wk", [D, D], BF16, isOutput=False)
    wv1_ext = nc.declare_dram_parameter("wv1", [D, D], BF16, isOutput=False)
    wv2_ext = nc.declare_dram_parameter("wv2", [D, D], BF16, isOutput=False)
    mask_ext = nc.declare_dram_parameter("masks", [N_MASKED, P, 256], BF16, isOutput=False)
    o_ext = nc.declare_dram_parameter("o", [NSLOTS, 2, P, D], F32, isOutput=True)

    # staging + gather buffers. Own-tile K^T payload is fp8-packed
    # [p, sub, pk, col] with d = sub*256 + pk*128 + p.
    klocal = nc.dram_tensor("klocal", [NGRP, P, 4, 2, 256], F8)
    vlocal = nc.dram_tensor("vlocal", [NGRP, 2, P, D], BF16)
    kgath = [
        nc.dram_tensor(f"kgath{g}", [NCORES, P, 4, 2, 256], F8, addr_space="Shared")
        for g in range(NGRP)
    ]
    vgath = [
        nc.dram_tensor(f"vgath{g}", [NCORES, 2, P, D], BF16, addr_space="Shared")
        for g in range(NGRP)
    ]
    RG = [list(range(NCORES))]

    with tile.TileContext(nc) as tc:
        singles_ctx = tc.tile_pool(name="singles", bufs=1)
        singles = singles_ctx.__enter__()

        # persistent fp8 K^T tiles + fp8 Q^T + dup V
        kt_grp = [
            singles.tile([P, 4, 2, 16 * P], F8, name=f"ktg{g}") for g in range(NGRP)
        ]
        kt_dup = singles.tile([P, 4, 2, NDUP * P], F8, name="ktdup")
        qt_sb = singles.tile([P, 4, 2, QT_COLS], F8, name="qtsb")
        v_dup = [singles.tile([P, D], BF16, name=f"vdup{j}") for j in range(NDUP)]

        ones_sb = singles.tile([P, 1], BF16)
        zcol_sb = singles.tile([1, P], BF16)
        zrow_sb = singles.tile([1, 512], BF16)
        ident_sb = singles.tile([P, P], BF16)

        with (
            tc.tile_pool(name="projw", bufs=1) as projw,
            tc.tile_pool(name="projout", bufs=4) as projout,
            tc.tile_pool(name="ppsum", bufs=4, space="PSUM") as ppsum,
        ):
            # chunked loads so the first projection matmuls start early
            wk_bf = projw.tile([P, 8, D], BF16, tag="wk", name="wk")
            wk_v = wk_ext[:].rearrange("(sub p) s -> p sub s", p=P)
            wv1_bf = projw.tile([P, 8, D], BF16, tag="wv1", name="wv1")
            wv1_v = wv1_ext[:].rearrange("(sub p) s -> p sub s", p=P)
            wq_bf = projw.tile([P, 8, D], BF16, tag="wq", name="wq")
            wq_v = wq_ext[:].rearrange("(sub p) s -> p sub s", p=P)
            xq_bf = projw.tile([P, 8, QT_COLS], BF16, tag="xq", name="xq")
            xq_v = xq_ext[:].rearrange("(sub p) s -> p sub s", p=P)
            xd_bf = projw.tile([P, 8, NDUP * P], BF16, tag="xd", name="xd")

            nc.sync.dma_start(out=xq_bf[:, :, :256], in_=xq_v[:, :, :256])
            for m in range(8):
                nc.sync.dma_start(
                    out=wk_bf[:, :, m * P : (m + 1) * P],
                    in_=wk_v[:, :, m * P : (m + 1) * P],
                )
            nc.sync.dma_start(out=xq_bf[:, :, 256:512], in_=xq_v[:, :, 256:512])
            for hh in range(2):
                nc.sync.dma_start(
                    out=wv1_bf[:, :, hh * 512 : (hh + 1) * 512],
                    in_=wv1_v[:, :, hh * 512 : (hh + 1) * 512],
                )
            nc.sync.dma_start(out=xq_bf[:, :, 512:], in_=xq_v[:, :, 512:])
            nc.sync.dma_start(out=wq_bf, in_=wq_v)
            nc.sync.dma_start(
                out=xd_bf, in_=xd_ext[:].rearrange("(sub p) s -> p sub s", p=P)
            )

            nc.vector.memset(ones_sb, 1.0)
            nc.vector.memset(zcol_sb, 0.0)
            nc.vector.memset(zrow_sb, 0.0)
            make_identity(nc, ident_sb)

            # ---- own K^T/V per group -> staging -> gathers -----------------
            for g in range(NGRP):
                c0 = g * 256
                # K^T own tile (256 cols), fp8-packed
                kc = projout.tile([P, 4, 2, 256], F8, tag="kc", name="kc")
                for m in range(8):
                    acc = ppsum.tile([P, 512], F32, tag="proj", name=f"kp{g}_{m}")
                    for sub in range(8):
                        nc.tensor.matmul(
                            acc[:, :256],
                            lhsT=wk_bf[:, sub, m * P : (m + 1) * P],
                            rhs=xq_bf[:, sub, c0 : c0 + 256],
                            start=(sub == 0),
                            stop=(sub == 7),
                        )
                    nc.vector.tensor_copy(out=kc[:, m // 2, m % 2, :], in_=acc[:, :256])
                nc.scalar.dma_start(out=klocal[g], in_=kc)
                # V own tile (2 blocks of 128 rows)
                for b in range(2):
                    vc = projout.tile([P, D], BF16, tag="vc", name="vc")
                    accs = [
                        ppsum.tile([P, 512], F32, tag="proj", name=f"vp{g}{b}{h}")
                        for h in range(2)
                    ]
                    for sub in range(8):
                        for h in range(2):
                            nc.tensor.matmul(
                                accs[h],
                                lhsT=xq_bf[:, sub, c0 + b * P : c0 + (b + 1) * P],
                                rhs=wv1_bf[:, sub, h * 512 : (h + 1) * 512],
                                start=(sub == 0),
                                stop=(sub == 7),
                            )
                    for h in range(2):
                        nc.vector.tensor_copy(
                            out=vc[:, h * 512 : (h + 1) * 512], in_=accs[h]
                        )
                    nc.scalar.dma_start(out=vlocal[g, b], in_=vc)
                nc.gpsimd.collective_compute(
                    "AllGather", mybir.AluOpType.bypass, replica_groups=RG,
                    ins=[klocal[g]], outs=[kgath[g][:]],
                )
                nc.gpsimd.collective_compute(
                    "AllGather", mybir.AluOpType.bypass, replica_groups=RG,
                    ins=[vlocal[g]], outs=[vgath[g][:]],
                )

            # gathered K^T -> resident SBUF (gpsimd queue: sits behind the
            # collective triggers, never blocks the sync/scalar streams)
            for g in range(NGRP):
                for gc in range(NCORES):
                    if g == 0 and gc < 2:
                        continue  # dup region, produced locally
                    nc.gpsimd.dma_start(
                        out=kt_grp[g][:, :, :, 256 * gc : 256 * (gc + 1)],
                        in_=kgath[g][gc],
                    )

            # ---- Q^T (fp8-packed) ------------------------------------------
            for m in range(8):
                accs = [
                    ppsum.tile([P, 512], F32, tag="proj", name=f"qp{m}{n}")
                    for n in range(2)
                ]
                for sub in range(8):
                    for n in range(2):
                        nc.tensor.matmul(
                            accs[n],
                            lhsT=wq_bf[:, sub, m * P : (m + 1) * P],
                            rhs=xq_bf[:, sub, n * 512 : (n + 1) * 512],
                            start=(sub == 0),
                            stop=(sub == 7),
                        )
                for n in range(2):
                    nc.vector.tensor_copy(
                        out=qt_sb[:, m // 2, m % 2, n * 512 : (n + 1) * 512],
                        in_=accs[n],
                    )

            # ---- dup prefix: K^T + V for kv blocks 0..NDUP-1 ---------------
            for m in range(8):
                acc = ppsum.tile([P, 512], F32, tag="proj", name=f"dk{m}")
                for sub in range(8):
                    nc.tensor.matmul(
                        acc,
                        lhsT=wk_bf[:, sub, m * P : (m + 1) * P],
                        rhs=xd_bf[:, sub, :],
                        start=(sub == 0),
                        stop=(sub == 7),
                    )
                nc.vector.tensor_copy(out=kt_dup[:, m // 2, m % 2, :], in_=acc)
            for blk in range(NDUP):
                accs = [
                    ppsum.tile([P, 512], F32, tag="proj", name=f"dv{blk}{h}")
                    for h in range(2)
                ]
                for sub in range(8):
                    for h in range(2):
                        nc.tensor.matmul(
                            accs[h],
                            lhsT=xd_bf[:, sub, blk * P : (blk + 1) * P],
                            rhs=wv1_bf[:, sub, h * 512 : (h + 1) * 512],
                            start=(sub == 0),
                            stop=(sub == 7),
                        )
                for h in range(2):
                    nc.vector.tensor_copy(
                        out=v_dup[blk][:, h * 512 : (h + 1) * 512], in_=accs[h]
                    )

        # ---- attention ----------------------------------------------------
        with (
            tc.tile_pool(name="asingles", bufs=1) as asingles,
            tc.tile_pool(name="vpool", bufs=8) as vpool,
            tc.tile_pool(name="mpool", bufs=4) as mpool,
            tc.tile_pool(name="epool", bufs=2) as epool,
            tc.tile_pool(name="gpool", bufs=2) as gpool,
            tc.tile_pool(name="ltpsum", bufs=2, space="PSUM") as ltpsum,
            tc.tile_pool(name="hpsum", bufs=1, space="PSUM") as hpsum,
            tc.tile_pool(name="mpsum", bufs=1, space="PSUM") as mpsum,
        ):
            wv2_bf = wload_attn(nc, asingles, wv2_ext)

            def kt_src(j):
                if j < NDUP:
                    return kt_dup, j * P
                g = j // 16
                return kt_grp[g], (j - 16 * g) * P

            def emit_visit(s, j):
                """DMA loads + logits for visit (s, j); returns pipe entry."""
                if j < NDUP:
                    v_t = v_dup[j]
                else:
                    g = j // 16
                    gc = (j // 2) % 8
                    v_t = vpool.tile([P, D], BF16, tag="v", name="v_t")
                    nc.sync.dma_start(out=v_t, in_=vgath[g][gc, j % 2])
                m_t = None
                if j >= 16 * s:
                    m_t = mpool.tile([P, 256], BF16, tag="m", name="m_t")
                    nc.scalar.dma_start(out=m_t, in_=mask_ext[j])
                lt = ltpsum.tile([P, 256], F32, tag="lt", name="lt")
                kt_t, c0 = kt_src(j)
                if use_dr:
                    for sub in range(4):
                        nc.tensor.matmul(
                            lt,
                            lhsT=kt_t[:, sub, :, c0 : c0 + P],
                            rhs=qt_sb[:, sub, :, s * 256 : (s + 1) * 256],
                            start=(sub == 0),
                            stop=(sub == 3),
                            perf_mode=DR,
                        )
                else:
                    for k, (sub, pk) in enumerate(
                        [(a, b) for a in range(4) for b in range(2)]
                    ):
                        nc.tensor.matmul(
                            lt,
                            lhsT=kt_t[:, sub, pk, c0 : c0 + P],
                            rhs=qt_sb[:, sub, pk, s * 256 : (s + 1) * 256],
                            start=(k == 0),
                            stop=(k == 7),
                        )
                return (j, lt, v_t, m_t)

            def pv(s, entry, h, sums, jmax):
                j, lt, v_t, m_t = entry
                pt = vpool.tile([P, 256], BF16, tag="pt", name="pt")
                nc.scalar.activation(out=pt, in_=lt, func=AF.Exp, scale=EXP_SCALE)
                if m_t is not None:
                    nc.vector.tensor_mul(out=pt, in0=pt, in1=m_t)
                for qc in range(2):
                    lhsT = pt[:, qc * P : (qc + 1) * P]
                    for dh in range(2):
                        nc.tensor.matmul(
                            h[qc][:, dh, :],
                            lhsT=lhsT,
                            rhs=v_t[:, dh * 512 : (dh + 1) * 512],
                            start=(j == 0),
                            stop=(j == jmax),
                        )
                    nc.tensor.matmul(
                        sums[:, qc : qc + 1],
                        lhsT=lhsT,
                        rhs=ones_sb,
                        start=False,
                        stop=(j == jmax),
                        skip_group_check=True,
                    )

            carry = []
            for s in range(NSLOTS):
                nv = 16 * (s + 1)
                jmax = nv - 1
                h = [
                    hpsum.tile([P, 2, 512], F32, tag=f"hq{qc}", name=f"h{qc}_{s}")
                    for qc in range(2)
                ]
                sums = mpsum.tile([P, 2], F32, tag="sums", name="sums")
                nc.tensor.matmul(
                    sums,
                    lhsT=zcol_sb,
                    rhs=zrow_sb[:, :2],
                    start=True,
                    stop=False,
                    skip_group_check=True,
                )
                pipe = list(carry)
                carry = []
                for j in range(len(pipe), LEAD):
                    pipe.append(emit_visit(s, j))
                for j in range(len(pipe), nv):
                    pv(s, pipe.pop(0), h, sums, jmax)
                    pipe.append(emit_visit(s, j))
                while pipe:
                    pv(s, pipe.pop(0), h, sums, jmax)

                # ---- epilogue part 1: consume h/sums (scalar+DVE only) ----
                g_bf = []
                for qc in range(2):
                    recip = epool.tile([P, 1], F32, tag="recip", name="recip")
                    nc.vector.reciprocal(out=recip, in_=sums[:, qc : qc + 1])
                    g32 = epool.tile([P, 2, 512], F32, tag=f"g32_{qc}", name="g32")
                    nc.vector.tensor_scalar_mul(out=g32, in0=h[qc], scalar1=recip)
                    gv = g32.rearrange("p a b -> p (a b)")
                    # silu(x) = x / (1 + exp(-x)) -- avoids an activation
                    # table swap between Exp and Sigmoid on the scalar engine
                    e = epool.tile([P, 1024], BF16, tag="e", name="e")
                    nc.scalar.activation(out=e, in_=gv, func=AF.Exp, scale=-1.0)
                    t = epool.tile([P, 1024], F32, tag="t", name="t")
                    nc.vector.tensor_scalar_add(out=t, in0=e, scalar1=1.0)
                    r = epool.tile([P, 1024], F32, tag="r", name="r")
                    nc.vector.reciprocal(out=r, in_=t)
                    g = gpool.tile([P, 1024], BF16, tag=f"g{qc}", name=f"g{qc}")
                    nc.vector.tensor_mul(out=g, in0=gv, in1=r)
                    g_bf.append(g)

                # lead visits of the next slot cover part-1 latency
                if s + 1 < NSLOTS:
                    for j in range(LEAD):
                        carry.append(emit_visit(s + 1, j))

                # ---- epilogue part 2: transpose G, output projection ------
                gt_sb = epool.tile([P, 8, 256], BF16, tag="gt", name="gt")
                for m in range(8):
                    for qc in range(2):
                        tp = mpsum.tile([P, 256], BF16, tag="tp", name="tp")
                        nc.tensor.transpose(
                            tp[:, :P],
                            g_bf[qc][:, m * P : (m + 1) * P],
                            ident_sb,
                        )
                        nc.vector.tensor_copy(
                            out=gt_sb[:, m, qc * P : (qc + 1) * P], in_=tp[:, :P]
                        )
                for qc in range(2):
                    op = hpsum.tile(
                        [P, 2, 512], F32, tag=f"hq{qc}", name=f"o{qc}_{s}"
                    )
                    for m in range(8):
                        for dh in range(2):
                            nc.tensor.matmul(
                                op[:, dh, :],
                                lhsT=gt_sb[:, m, qc * P : (qc + 1) * P],
                                rhs=wv2_bf[:, m, dh * 512 : (dh + 1) * 512],
                                start=(m == 0),
                                stop=(m == 7),
                            )
                    oo = epool.tile([P, 2, 512], F32, tag="oo", name="oo")
                    nc.vector.tensor_copy(out=oo, in_=op)
                    nc.sync.dma_start(
                        out=o_ext[s, qc], in_=oo.rearrange("p a b -> p (a b)")
                    )

        singles_ctx.__exit__(None, None, None)

    nc.finalize()
    return nc


def wload_attn(nc, pool, ext):
    t = pool.tile([P, 8, D], BF16, tag="wv2", name="wv2")
    nc.sync.dma_start(out=t, in_=ext[:].rearrange("(sub p) s -> p sub s", p=P))
    return t


_NC_CACHE = {}


def get_nc(use_dr=USE_DR):
    if use_dr not in _NC_CACHE:
        _NC_CACHE[use_dr] = build_kernel(use_dr)
    return _NC_CACHE[use_dr]


def build_masks():
    """Masks for the last 16 visits of each slot, selected per core by
    k = 2c + 16s - j: k>=1 all-visible, k==0 upper-left triangle, k==-1
    shifted triangle, k<=-2 fully masked (padded visit)."""
    p = np.arange(P)[:, None]
    u = np.arange(256)[None, :]
    m_ones = np.ones((P, 256), np.float32)
    m0 = (p <= u).astype(np.float32)
    m1 = (p <= u - P).astype(np.float32)
    m_zero = np.zeros((P, 256), np.float32)
    canon = np.stack([m_zero, m1, m0, m_ones]).astype(ml_dtypes.bfloat16)

    out = []
    for c in range(NCORES):
        sel = []
        for s in range(NSLOTS):
            for j in range(16 * s, 16 * (s + 1)):
                k = 2 * c + 16 * s - j
                sel.append(min(max(k, -2), 1) + 2)
        out.append(canon[np.array(sel, np.int64)])
    return out  # list of [64, 128, 256] bf16


def build_in_maps(x, wq, wk, wv1, wv2):
    bf = ml_dtypes.bfloat16
    xT = np.ascontiguousarray(np.asarray(x, np.float32).T).astype(bf)
    masks = build_masks()
    xd = np.ascontiguousarray(xT[:, : NDUP * P])
    w = {
        "wq": (np.asarray(wq, np.float32) * QK_SCALE).astype(bf),
        "wk": (np.asarray(wk, np.float32) * QK_SCALE).astype(bf),
        "wv1": np.asarray(wv1, np.float32).astype(bf),
        "wv2": np.asarray(wv2, np.float32).astype(bf),
    }
    in_maps = []
    for c in range(NCORES):
        xq_c = np.concatenate(
            [xT[:, 256 * (c + 8 * s) : 256 * (c + 8 * s) + 256] for s in range(NSLOTS)],
            axis=1,
        )
        in_maps.append(
            {"xq": np.ascontiguousarray(xq_c), "xd": xd, "masks": masks[c], **w}
        )
    return in_maps


def assemble_out(results):
    out = np.empty((SEQ, D), np.float32)
    for c in range(NCORES):
        o = results[c]["o"]  # [4, 2, 128, 1024]
        for s in range(NSLOTS):
            r0 = 256 * (c + 8 * s)
            out[r0 : r0 + P, :] = o[s, 0]
            out[r0 + P : r0 + 256, :] = o[s, 1]
    return out


def kernel(x, wq, wk, wv1, wv2):
    in_maps = build_in_maps(x, wq, wk, wv1, wv2)
    nc = get_nc()
    res = run_bass_kernel_spmd(nc, in_maps, list(range(NCORES)))
    return assemble_out(res.results)


# revision 8
# speedup vs baseline: 1.1096x; 1.1096x over previous
"""Trainium2 Bass kernel for nn_MemoryAttention (causal single-head attention
with SiLU-gated output projection), sequence-parallel across 8 NeuronCores.

Strategy (per core c), v3:
  - q rows owned: 4 slots of 256 rows: tile t = c + 8*s (strided assignment
    balances causal work; every core runs an identical instruction stream).
  - K^T is kept SBUF-resident in fp8e4 (packed [d/256, 2, kv] for DoubleRow
    matmuls); Q^T likewise fp8-packed. wq/wk are pre-scaled x8 on the host so
    q/k values sit in fp8e4's normal range; the exp() activation scale absorbs
    the x64 on the logits.
  - KV distribution: 4 gather groups of 16 kv blocks each. Core c projects
    K^T/V only for its own tile in each group, then (K_g, V_g) AllGathers are
    triggered progressively (K before V so logits can start first).
  - kv blocks 0..3 are projected redundantly on every core (dup prefix) so
    slot 0 can start before gather A lands. V for the dup prefix stays in
    SBUF; gathered V streams from shared DRAM per visit (bf16).
  - Per (slot, kv-block) visit: LT[kv, q] = K @ QT accumulated in PSUM via
    4 DoubleRow fp8 matmuls (256-deep contraction each), PT = exp(LT*scale)
    (* mask for the last 16 visits of each slot), then PT q-chunks become the
    stationary operand for H[q, d] += P @ V and rowsums += P @ 1. The visit
    pipeline runs 2 deep and carries across slot boundaries so the PE stays
    busy through the epilogue's scalar/vector latency.
  - Slot epilogue: H / sums, SiLU via exp (no activation-table swap:
    silu(x) = x / (1 + exp(-x))), PE-transpose of G, output projection with
    G^T chunks stationary -> O[q, d] written directly.
  - Queues: visit V streams on sync; masks + staging on scalar (HWDGE);
    collectives + bulk K^T gather->SBUF loads on gpsimd.
"""

import numpy as np
import ml_dtypes

import concourse.bass as bass
import concourse.tile as tile
from concourse import bacc, mybir
from concourse.bass_utils import run_bass_kernel_spmd
from concourse.masks import make_identity

P = 128
D = 1024
SEQ = 8192
NCORES = 8
NSLOTS = 4
QT_COLS = NSLOTS * 256
NDUP = 4              # dup kv blocks, cols 0..NDUP*128
NGRP = 4              # gather groups of 16 kv blocks
N_MASKED = NSLOTS * 16
LEAD = 2              # visit software-pipeline depth

F32 = mybir.dt.float32
BF16 = mybir.dt.bfloat16
F8 = mybir.dt.float8e4
AF = mybir.ActivationFunctionType
DR = mybir.MatmulPerfMode.DoubleRow

QK_SCALE = 8.0
EXP_SCALE = 0.03125 / (QK_SCALE * QK_SCALE)

USE_DR = True


def build_kernel(use_dr=USE_DR):
    nc = bacc.Bacc(None, target_bir_lowering=False, num_devices=NCORES)

    xq_ext = nc.declare_dram_parameter("xq", [D, QT_COLS], BF16, isOutput=False)
    xd_ext = nc.declare_dram_parameter("xd", [D, NDUP * P], BF16, isOutput=False)
    wq_ext = nc.declare_dram_parameter("wq", [D, D], BF16, isOutput=False)
    wk_ext = nc.declare_dram_parameter("wk", [D, D], BF16, isOutput=False)
    wv1_ext = nc.declare_dram_parameter("wv1", [D, D], BF16, isOutput=False)
    wv2_ext = nc.declare_dram_parameter("wv2", [D, D], BF16, isOutput=False)
    mask_ext = nc.declare_dram_parameter("masks", [N_MASKED, P, 256], BF16, isOutput=False)
    o_ext = nc.declare_dram_parameter("o", [NSLOTS, 2, P, D], F32, isOutput=True)

    # staging + gather buffers. Own-tile K^T payload is fp8-packed
    # [p, sub, pk, col] with d = sub*256 + pk*128 + p.
    klocal = nc.dram_tensor("klocal", [NGRP, P, 4, 2, 256], F8)
    vlocal = nc.dram_tensor("vlocal", [NGRP, 2, P, D], BF16)
    kgath = [
        nc.dram_tensor(f"kgath{g}", [NCORES, P, 4, 2, 256], F8, addr_space="Shared")
        for g in range(NGRP)
    ]
    vgath = [
        nc.dram_tensor(f"vgath{g}", [NCORES, 2, P, D], BF16, addr_space="Shared")
        for g in range(NGRP)
    ]
    RG = [list(range(NCORES))]

    with tile.TileContext(nc) as tc:
        singles_ctx = tc.tile_pool(name="singles", bufs=1)
        singles = singles_ctx.__enter__()

        # persistent fp8 K^T tiles + fp8 Q^T + dup V
        kt_grp = [
            singles.tile([P, 4, 2, 16 * P], F8, name=f"ktg{g}") for g in range(NGRP)
        ]
        kt_dup = singles.tile([P, 4, 2, NDUP * P], F8, name="ktdup")
        qt_sb = singles.tile([P, 4, 2, QT_COLS], F8, name="qtsb")
        v_dup = [singles.tile([P, D], BF16, name=f"vdup{j}") for j in range(NDUP)]

        ones_sb = singles.tile([P, 1], BF16)
        zcol_sb = singles.tile([1, P], BF16)
        zrow_sb = singles.tile([1, 512], BF16)
        ident_sb = singles.tile([P, P], BF16)

        with (
            tc.tile_pool(name="projw", bufs=1) as projw,
            tc.tile_pool(name="projout", bufs=4) as projout,
            tc.tile_pool(name="ppsum", bufs=4, space="PSUM") as ppsum,
        ):
            # chunked loads so the first projection matmuls start early
            wk_bf = projw.tile([P, 8, D], BF16, tag="wk", name="wk")
            wk_v = wk_ext[:].rearrange("(sub p) s -> p sub s", p=P)
            wv1_bf = projw.tile([P, 8, D], BF16, tag="wv1", name="wv1")
            wv1_v = wv1_ext[:].rearrange("(sub p) s -> p sub s", p=P)
            wq_bf = projw.tile([P, 8, D], BF16, tag="wq", name="wq")
            wq_v = wq_ext[:].rearrange("(sub p) s -> p sub s", p=P)
            xq_bf = projw.tile([P, 8, QT_COLS], BF16, tag="xq", name="xq")
            xq_v = xq_ext[:].rearrange("(sub p) s -> p sub s", p=P)
            xd_bf = projw.tile([P, 8, NDUP * P], BF16, tag="xd", name="xd")

            nc.sync.dma_start(out=xq_bf[:, :, :256], in_=xq_v[:, :, :256])
            for m in range(8):
                nc.sync.dma_start(
                    out=wk_bf[:, :, m * P : (m + 1) * P],
                    in_=wk_v[:, :, m * P : (m + 1) * P],
                )
            nc.sync.dma_start(out=xq_bf[:, :, 256:512], in_=xq_v[:, :, 256:512])
            for hh in range(2):
                nc.sync.dma_start(
                    out=wv1_bf[:, :, hh * 512 : (hh + 1) * 512],
                    in_=wv1_v[:, :, hh * 512 : (hh + 1) * 512],
                )
            nc.sync.dma_start(out=xq_bf[:, :, 512:], in_=xq_v[:, :, 512:])
            nc.sync.dma_start(out=wq_bf, in_=wq_v)
            nc.sync.dma_start(
                out=xd_bf, in_=xd_ext[:].rearrange("(sub p) s -> p sub s", p=P)
            )

            nc.vector.memset(ones_sb, 1.0)
            nc.vector.memset(zcol_sb, 0.0)
            nc.vector.memset(zrow_sb, 0.0)
            make_identity(nc, ident_sb)

            # ---- own K^T/V per group -> staging -> gathers -----------------
            for g in range(NGRP):
                c0 = g * 256
                # K^T own tile (256 cols), fp8-packed
                kc = projout.tile([P, 4, 2, 256], F8, tag="kc", name="kc")
                for m in range(8):
                    acc = ppsum.tile([P, 512], F32, tag="proj", name=f"kp{g}_{m}")
                    for sub in range(8):
                        nc.tensor.matmul(
                            acc[:, :256],
                            lhsT=wk_bf[:, sub, m * P : (m + 1) * P],
                            rhs=xq_bf[:, sub, c0 : c0 + 256],
                            start=(sub == 0),
                            stop=(sub == 7),
                        )
                    nc.vector.tensor_copy(out=kc[:, m // 2, m % 2, :], in_=acc[:, :256])
                nc.scalar.dma_start(out=klocal[g], in_=kc)
                # V own tile (2 blocks of 128 rows)
                for b in range(2):
                    vc = projout.tile([P, D], BF16, tag="vc", name="vc")
                    accs = [
                        ppsum.tile([P, 512], F32, tag="proj", name=f"vp{g}{b}{h}")
                        for h in range(2)
                    ]
                    for sub in range(8):
                        for h in range(2):
                            nc.tensor.matmul(
                                accs[h],
                                lhsT=xq_bf[:, sub, c0 + b * P : c0 + (b + 1) * P],
                                rhs=wv1_bf[:, sub, h * 512 : (h + 1) * 512],
                                start=(sub == 0),
                                stop=(sub == 7),
                            )
                    for h in range(2):
                        nc.vector.tensor_copy(
                            out=vc[:, h * 512 : (h + 1) * 512], in_=accs[h]
                        )
                    nc.scalar.dma_start(out=vlocal[g, b], in_=vc)
                nc.gpsimd.collective_compute(
                    "AllGather", mybir.AluOpType.bypass, replica_groups=RG,
                    ins=[klocal[g]], outs=[kgath[g][:]],
                )
                nc.gpsimd.collective_compute(
                    "AllGather", mybir.AluOpType.bypass, replica_groups=RG,
                    ins=[vlocal[g]], outs=[vgath[g][:]],
                )

            # gathered K^T -> resident SBUF (gpsimd queue: sits behind the
            # collective triggers, never blocks the sync/scalar streams)
            for g in range(NGRP):
                for gc in range(NCORES):
                    if g == 0 and gc < 2:
                        continue  # dup region, produced locally
                    nc.gpsimd.dma_start(
                        out=kt_grp[g][:, :, :, 256 * gc : 256 * (gc + 1)],
                        in_=kgath[g][gc],
                    )

            # ---- Q^T (fp8-packed) ------------------------------------------
            for m in range(8):
                accs = [
                    ppsum.tile([P, 512], F32, tag="proj", name=f"qp{m}{n}")
                    for n in range(2)
                ]
                for sub in range(8):
                    for n in range(2):
                        nc.tensor.matmul(
                            accs[n],
                            lhsT=wq_bf[:, sub, m * P : (m + 1) * P],
                            rhs=xq_bf[:, sub, n * 512 : (n + 1) * 512],
                            start=(sub == 0),
                            stop=(sub == 7),
                        )
                for n in range(2):
                    nc.vector.tensor_copy(
                        out=qt_sb[:, m // 2, m % 2, n * 512 : (n + 1) * 512],
                        in_=accs[n],
                    )

            # ---- dup prefix: K^T + V for kv blocks 0..NDUP-1 ---------------
            for m in range(8):
                acc = ppsum.tile([P, 512], F32, tag="proj", name=f"dk{m}")
                for sub in range(8):
                    nc.tensor.matmul(
                        acc,
                        lhsT=wk_bf[:, sub, m * P : (m + 1) * P],
                        rhs=xd_bf[:, sub, :],
                        start=(sub == 0),
                        stop=(sub == 7),
                    )
                nc.vector.tensor_copy(out=kt_dup[:, m // 2, m % 2, :], in_=acc)
            for blk in range(NDUP):
                accs = [
                    ppsum.tile([P, 512], F32, tag="proj", name=f"dv{blk}{h}")
                    for h in range(2)
                ]
                for sub in range(8):
                    for h in range(2):
                        nc.tensor.matmul(
                            accs[h],
                            lhsT=xd_bf[:, sub, blk * P : (blk + 1) * P],
                            rhs=wv1_bf[:, sub, h * 512 : (h + 1) * 512],
                            start=(sub == 0),
                            stop=(sub == 7),
                        )
                for h in range(2):
                    nc.vector.tensor_copy(
                        out=v_dup[blk][:, h * 512 : (h + 1) * 512], in_=accs[h]
                    )

        # ---- attention ----------------------------------------------------
        with (
            tc.tile_pool(name="asingles", bufs=1) as asingles,
            tc.tile_pool(name="vpool", bufs=8) as vpool,
            tc.tile_pool(name="mpool", bufs=4) as mpool,
            tc.tile_pool(name="epool", bufs=2) as epool,
            tc.tile_pool(name="gpool", bufs=2) as gpool,
            tc.tile_pool(name="ltpsum", bufs=2, space="PSUM") as ltpsum,
            tc.tile_pool(name="hpsum", bufs=1, space="PSUM") as hpsum,
            tc.tile_pool(name="mpsum", bufs=1, space="PSUM") as mpsum,
        ):
            wv2_bf = wload_attn(nc, asingles, wv2_ext)

            def kt_src(j):
                if j < NDUP:
                    return kt_dup, j * P
                g = j // 16
                return kt_grp[g], (j - 16 * g) * P

            def emit_visit(s, j):
                """DMA loads + logits for visit (s, j); returns pipe entry."""
                if j < NDUP:
                    v_t = v_dup[j]
                else:
                    g = j // 16
                    gc = (j // 2) % 8
                    v_t = vpool.tile([P, D], BF16, tag="v", name="v_t")
                    nc.sync.dma_start(out=v_t, in_=vgath[g][gc, j % 2])
                m_t = None
                if j >= 16 * s:
                    m_t = mpool.tile([P, 256], BF16, tag="m", name="m_t")
                    nc.scalar.dma_start(out=m_t, in_=mask_ext[j])
                lt = ltpsum.tile([P, 256], F32, tag="lt", name="lt")
                kt_t, c0 = kt_src(j)
                if use_dr:
                    for sub in range(4):
                        nc.tensor.matmul(
                            lt,
                            lhsT=kt_t[:, sub, :, c0 : c0 + P],
                            rhs=qt_sb[:, sub, :, s * 256 : (s + 1) * 256],
                            start=(sub == 0),
                            stop=(sub == 3),
                            perf_mode=DR,
                        )
                else:
                    for k, (sub, pk) in enumerate(
                        [(a, b) for a in range(4) for b in range(2)]
                    ):
                        nc.tensor.matmul(
                            lt,
                            lhsT=kt_t[:, sub, pk, c0 : c0 + P],
                            rhs=qt_sb[:, sub, pk, s * 256 : (s + 1) * 256],
                            start=(k == 0),
                            stop=(k == 7),
                        )
                return (j, lt, v_t, m_t)

            def pv(s, entry, h, sums, jmax):
                j, lt, v_t, m_t = entry
                pt = vpool.tile([P, 256], BF16, tag="pt", name="pt")
                nc.scalar.activation(out=pt, in_=lt, func=AF.Exp, scale=EXP_SCALE)
                if m_t is not None:
                    nc.vector.tensor_mul(out=pt, in0=pt, in1=m_t)
                for qc in range(2):
                    lhsT = pt[:, qc * P : (qc + 1) * P]
                    for dh in range(2):
                        nc.tensor.matmul(
                            h[qc][:, dh, :],
                            lhsT=lhsT,
                            rhs=v_t[:, dh * 512 : (dh + 1) * 512],
                            start=(j == 0),
                            stop=(j == jmax),
                        )
                    nc.tensor.matmul(
                        sums[:, qc : qc + 1],
                        lhsT=lhsT,
                        rhs=ones_sb,
                        start=False,
                        stop=(j == jmax),
                        skip_group_check=True,
                    )

            carry = []
            for s in range(NSLOTS):
                nv = 16 * (s + 1)
                jmax = nv - 1
                h = [
                    hpsum.tile([P, 2, 512], F32, tag=f"hq{qc}", name=f"h{qc}_{s}")
                    for qc in range(2)
                ]
                sums = mpsum.tile([P, 2], F32, tag="sums", name="sums")
                nc.tensor.matmul(
                    sums,
                    lhsT=zcol_sb,
                    rhs=zrow_sb[:, :2],
                    start=True,
                    stop=False,
                    skip_group_check=True,
                )
                pipe = list(carry)
                carry = []
                for j in range(len(pipe), LEAD):
                    pipe.append(emit_visit(s, j))
                for j in range(len(pipe), nv):
                    pv(s, pipe.pop(0), h, sums, jmax)
                    pipe.append(emit_visit(s, j))
                while pipe:
                    pv(s, pipe.pop(0), h, sums, jmax)

                # ---- epilogue part 1: consume h/sums (scalar+DVE only) ----
                g_bf = []
                for qc in range(2):
                    recip = epool.tile([P, 1], F32, tag="recip", name="recip")
                    nc.vector.reciprocal(out=recip, in_=sums[:, qc : qc + 1])
                    g32 = epool.tile([P, 2, 512], F32, tag=f"g32_{qc}", name="g32")
                    nc.vector.tensor_scalar_mul(out=g32, in0=h[qc], scalar1=recip)
                    gv = g32.rearrange("p a b -> p (a b)")
                    sg = epool.tile([P, 1024], BF16, tag="sg", name="sg")
                    nc.scalar.activation(out=sg, in_=gv, func=AF.Sigmoid)
                    g = gpool.tile([P, 1024], BF16, tag=f"g{qc}", name=f"g{qc}")
                    nc.vector.tensor_mul(out=g, in0=gv, in1=sg)
                    g_bf.append(g)

                # lead visits of the next slot cover part-1 latency
                if s + 1 < NSLOTS:
                    for j in range(LEAD):
                        carry.append(emit_visit(s + 1, j))

                # ---- epilogue part 2: transpose G, output projection ------
                gt_sb = epool.tile([P, 8, 256], BF16, tag="gt", name="gt")
                for m in range(8):
                    for qc in range(2):
                        tp = mpsum.tile([P, 256], BF16, tag="tp", name="tp")
                        nc.tensor.transpose(
                            tp[:, :P],
                            g_bf[qc][:, m * P : (m + 1) * P],
                            ident_sb,
                        )
                        nc.vector.tensor_copy(
                            out=gt_sb[:, m, qc * P : (qc + 1) * P], in_=tp[:, :P]
                        )
                for qc in range(2):
                    op = hpsum.tile(
                        [P, 2, 512], F32, tag=f"hq{qc}", name=f"o{qc}_{s}"
                    )
                    for m in range(8):
                        for dh in range(2):
                            nc.tensor.matmul(
                                op[:, dh, :],
                                lhsT=gt_sb[:, m, qc * P : (qc + 1) * P],
                                rhs=wv2_bf[:, m, dh * 512 : (dh + 1) * 512],
                                start=(m == 0),
                                stop=(m == 7),
                            )
                    oo = epool.tile([P, 2, 512], F32, tag="oo", name="oo")
                    nc.vector.tensor_copy(out=oo, in_=op)
                    nc.sync.dma_start(
                        out=o_ext[s, qc], in_=oo.rearrange("p a b -> p (a b)")
                    )

        singles_ctx.__exit__(None, None, None)

    nc.finalize()
    return nc


def wload_attn(nc, pool, ext):
    t = pool.tile([P, 8, D], BF16, tag="wv2", name="wv2")
    nc.sync.dma_start(out=t, in_=ext[:].rearrange("(sub p) s -> p sub s", p=P))
    return t


_NC_CACHE = {}


def get_nc(use_dr=USE_DR):
    if use_dr not in _NC_CACHE:
        _NC_CACHE[use_dr] = build_kernel(use_dr)
    return _NC_CACHE[use_dr]


def build_masks():
    """Masks for the last 16 visits of each slot, selected per core by
    k = 2c + 16s - j: k>=1 all-visible, k==0 upper-left triangle, k==-1
    shifted triangle, k<=-2 fully masked (padded visit)."""
    p = np.arange(P)[:, None]
    u = np.arange(256)[None, :]
    m_ones = np.ones((P, 256), np.float32)
    m0 = (p <= u).astype(np.float32)
    m1 = (p <= u - P).astype(np.float32)
    m_zero = np.zeros((P, 256), np.float32)
    canon = np.stack([m_zero, m1, m0, m_ones]).astype(ml_dtypes.bfloat16)

    out = []
    for c in range(NCORES):
        sel = []
        for s in range(NSLOTS):
            for j in range(16 * s, 16 * (s + 1)):
                k = 2 * c + 16 * s - j
                sel.append(min(max(k, -2), 1) + 2)
        out.append(canon[np.array(sel, np.int64)])
    return out  # list of [64, 128, 256] bf16


def build_in_maps(x, wq, wk, wv1, wv2):
    bf = ml_dtypes.bfloat16
    xT = np.ascontiguousarray(np.asarray(x, np.float32).T).astype(bf)
    masks = build_masks()
    xd = np.ascontiguousarray(xT[:, : NDUP * P])
    w = {
        "wq": (np.asarray(wq, np.float32) * QK_SCALE).astype(bf),
        "wk": (np.asarray(wk, np.float32) * QK_SCALE).astype(bf),
        "wv1": np.asarray(wv1, np.float32).astype(bf),
        "wv2": np.asarray(wv2, np.float32).astype(bf),
    }
    in_maps = []
    for c in range(NCORES):
        xq_c = np.concatenate(
            [xT[:, 256 * (c + 8 * s) : 256 * (c + 8 * s) + 256] for s in range(NSLOTS)],
            axis=1,
        )
        in_maps.append(
            {"xq": np.ascontiguousarray(xq_c), "xd": xd, "masks": masks[c], **w}
        )
    return in_maps


def assemble_out(results):
    out = np.empty((SEQ, D), np.float32)
    for c in range(NCORES):
        o = results[c]["o"]  # [4, 2, 128, 1024]
        for s in range(NSLOTS):
            r0 = 256 * (c + 8 * s)
            out[r0 : r0 + P, :] = o[s, 0]
            out[r0 + P : r0 + 256, :] = o[s, 1]
    return out


def kernel(x, wq, wk, wv1, wv2):
    in_maps = build_in_maps(x, wq, wk, wv1, wv2)
    nc = get_nc()
    res = run_bass_kernel_spmd(nc, in_maps, list(range(NCORES)))
    return assemble_out(res.results)
